# revision 10
# baseline (speedup 1.0000x reference)
"""Multi-head self-attention Trainium2 Bass kernel.

Problem: y = (softmax((x@Wq)(x@Wk)^T / sqrt(hd)) (x@Wv)) @ Wp + biases
with B=4, T=2048, C=1024, H=16, hd=64.

Sharding over 8 NeuronCores: (batch b in 0..3) x (head-group g in 0..1, 8
heads each) — tensor-parallel over heads, data-parallel over batch.  Each
core computes the attention for its batch and head group plus the partial
output projection restricted to its head group's features; the host sums
the two head-group partials per batch (the row-parallel TP reduction) and
transposes back.

Per-core device layout (T=2048, C=1024, Cg=512):
  x_t   [C, T]   f32  host-transposed activations
  Q^T,K^T [feat, tok] f32 computed as (w_qk chunk)^T @ x_t chunk
  V     [tok, feat]  bf16 computed as (x_t chunk)^T @ w_v chunk
  scores^T [k, q] = K_h Q_h^T / 8 accumulated in PSUM (contract d=64)
  E = exp(scores^T) via ScalarE with fused 0.125 scale, bf16
  U^T[d,q] (+ denominator rows) = [V_h | ones]^T @ E via two col-tiled
    matmuls into one PSUM tile; normalize with DVE reciprocal+mult
  out^T[oc, tok] = (Y @ w_p)^T + b_out, DMA'd out; host sums pairs.
"""

import os

import numpy as np
import ml_dtypes

import concourse.bass as bass
import concourse.bacc as bacc
import concourse.tile as tile
from concourse import mybir
from concourse.bass_utils import run_bass_kernel_spmd

N_CORES = 8
C = 1024           # embed dim
H = 16             # total heads
HD = 64            # head dim
HPC = 8            # heads per core
CG = HPC * HD      # 512: per-core q/k/v feature width

F32 = mybir.dt.float32
F32R = mybir.dt.float32r
BF16 = mybir.dt.bfloat16


def _body(tc, T, x_t, w_qk, b_qk, w_v, ones_bf, w_p, b_out, out_t):
    nc = tc.nc
    KC = C // 128           # contraction chunks over C
    FC = 2 * CG // 128      # q||k feature chunks
    TC4 = T // 512          # token chunks of 512
    TC1 = T // 128          # key chunks of 128
    QW = min(1024, T)       # q block width in stage 2
    QCH = T // QW
    OCC = C // 128          # output channel chunks
    PCH = CG // 128         # proj contraction chunks (4)
    Exp = mybir.ActivationFunctionType.Exp
    Mult = mybir.AluOpType.mult

    with (
        tc.tile_pool(name="const", bufs=1) as constp,
        tc.tile_pool(name="persist", bufs=1) as pers,
    ):
        ones_col = constp.tile([128, 8], BF16, tag="ones_col")
        nc.sync.dma_start(ones_col[:], ones_bf[:])
        bqk_sb = constp.tile([128, FC], F32, tag="bqk")
        nc.sync.dma_start(bqk_sb[:], b_qk[:])
        bout_sb = constp.tile([128, OCC], F32, tag="bout")
        nc.sync.dma_start(bout_sb[:], b_out[:])

        xt = []
        for i in range(KC):
            t = pers.tile([128, T], F32R, tag=f"xt{i}")
            nc.sync.dma_start(t[:], x_t[i * 128:(i + 1) * 128, :])
            xt.append(t)

        # per-head blocks of 65 cols: [V_h (64) | ones (1)] so one M=65
        # matmul produces U^T rows 0:64 and the softmax denominator row 64.
        v2 = [pers.tile([128, CG + HPC], BF16, tag=f"v2_{i}", name=f"v2_{i}")
              for i in range(TC1)]
        qkt = [pers.tile([128, T], F32R, tag=f"qkt{i}", name=f"qkt{i}")
               for i in range(FC)]
        yt = [pers.tile([128, T], BF16, tag=f"yt{i}", name=f"yt{i}")
              for i in range(PCH)]

        # ---- stage 1a: V = x @ w_v + b_v in [tok, feat] layout, bf16
        with (
            tc.tile_pool(name="wv", bufs=1) as wvp,
            tc.tile_pool(name="ps1", bufs=4, space="PSUM") as ps1,
        ):
            wv = []
            for kc in range(KC):
                t = wvp.tile([128, CG], F32R, tag=f"wv{kc}")
                nc.sync.dma_start(t[:], w_v[kc * 128:(kc + 1) * 128, :])
                wv.append(t)
            for tokc in range(TC1):
                vps = ps1.tile([128, CG], F32, tag="vps")
                for kc in range(KC):
                    nc.tensor.matmul(
                        vps[:], xt[kc][:, tokc * 128:(tokc + 1) * 128], wv[kc][:],
                        start=(kc == 0), stop=(kc == KC - 1))
                v2v = v2[tokc].rearrange("p (h c) -> p h c", c=HD + 1)
                nc.vector.tensor_copy(v2v[:, :, 0:HD],
                                      vps.rearrange("p (h c) -> p h c", c=HD))
                nc.sync.dma_start(v2v[:, :, HD], ones_bf[:])

            # ---- stage 1b: QK^T = (x @ w_qk)^T + b in [feat, tok], f32
            # fc order pairs each Q chunk with its K chunk so stage 2's
            # first head pair can start after 2/8 of this stage.
            with tc.tile_pool(name="wqk", bufs=2) as wqkp:
                for fc in [x for p in range(FC // 2) for x in (p, FC // 2 + p)]:
                    wts = []
                    for kc in range(KC):
                        t = wqkp.tile([128, 128], F32R, tag=f"wqk{kc}")
                        nc.sync.dma_start(
                            t[:],
                            w_qk[kc * 128:(kc + 1) * 128, fc * 128:(fc + 1) * 128])
                        wts.append(t)
                    for t4 in range(TC4):
                        ps = ps1.tile([128, 512], F32, tag="qkps")
                        for kc in range(KC):
                            nc.tensor.matmul(
                                ps[:], wts[kc][:],
                                xt[kc][:, t4 * 512:(t4 + 1) * 512],
                                start=(kc == 0), stop=(kc == KC - 1))
                        nc.vector.tensor_scalar_add(
                            qkt[fc][:, t4 * 512:(t4 + 1) * 512], ps[:],
                            bqk_sb[:, fc:fc + 1])

        # ---- stage 2: head PAIRS, scores^T -> exp -> AV(+denominator) -> Y^T
        # The two heads of a pair sit at partitions 0:64 / 64:128 of one
        # qkt chunk, so their score matmuls (K=64) land on disjoint PE row
        # groups and run concurrently; the AV matmul and its ones-lhsT
        # denominator twin are col-tiled (M=64 each) and also concurrent.
        with (
            tc.tile_pool(name="e", bufs=3) as ep,
            tc.tile_pool(name="rec", bufs=2) as recp,
            tc.tile_pool(name="recb", bufs=2) as recbp,
            tc.tile_pool(name="sps", bufs=2, space="PSUM") as spsp,
            tc.tile_pool(name="ups", bufs=2, space="PSUM") as upsp,
        ):
            for pair in range(HPC // 2):
                qt, kt = qkt[pair], qkt[FC // 2 + pair]
                for qc in range(T // 512):
                    q0 = qc * 512
                    upsAB = [upsp.tile([128, 512], F32, tag=f"ups{s}",
                                       name=f"ups{s}_{pair}_{qc}")
                             for s in "AB"]
                    for kc in range(TC1):
                        first, last = (kc == 0), (kc == TC1 - 1)
                        ets = []
                        for s in range(2):
                            po = s * 64
                            sps = spsp.tile([128, 512], F32, tag=f"sps{s}",
                                            name=f"sps{s}_{pair}_{qc}_{kc}")
                            nc.tensor.matmul(
                                sps[:],
                                kt[po:po + 64, kc * 128:(kc + 1) * 128],
                                qt[po:po + 64, q0:q0 + 512],
                                start=True, stop=True)
                            et = ep.tile([128, 512], BF16, tag=f"et{s}",
                                         name=f"et{s}_{pair}_{qc}_{kc}")
                            nc.scalar.activation(et[:], sps[:], Exp,
                                                 scale=0.125)
                            ets.append(et)
                        for s in range(2):
                            h = 2 * pair + s
                            ej, ups = ets[s], upsAB[s]
                            nc.tensor.matmul(
                                ups[0:65, :],
                                v2[kc][:, h * 65:(h + 1) * 65], ej[:],
                                start=first, stop=last)
                    for s in range(2):
                        po, ups = s * 64, upsAB[s]
                        rec = recp.tile([1, 512], F32, tag=f"rec{s}",
                                        name=f"rec{s}_{pair}_{qc}")
                        nc.vector.reciprocal(rec[:], ups[64:65, :])
                        recb = recbp.tile([64, 512], F32, tag=f"recb{s}",
                                          name=f"recb{s}_{pair}_{qc}")
                        nc.gpsimd.partition_broadcast(recb[:], rec[:])
                        nc.vector.tensor_tensor(
                            yt[pair][po:po + 64, q0:q0 + 512],
                            ups[0:64, :], recb[:], op=Mult)

        # ---- stage 3: out^T = (Y @ w_p)^T + b_out
        with (
            tc.tile_pool(name="wp", bufs=1) as wpp,
            tc.tile_pool(name="outp", bufs=2) as outp,
            tc.tile_pool(name="ps3", bufs=4, space="PSUM") as ps3,
        ):
            wp = []
            for fcp in range(PCH):
                t = wpp.tile([128, C], BF16, tag=f"wp{fcp}")
                nc.sync.dma_start(t[:], w_p[fcp * 128:(fcp + 1) * 128, :])
                wp.append(t)
            for occ in range(OCC):
                osb = outp.tile([128, T], F32, tag="osb")
                for t4 in range(TC4):
                    ps = ps3.tile([128, 512], F32, tag="ps3")
                    for fcp in range(PCH):
                        nc.tensor.matmul(
                            ps[:], wp[fcp][:, occ * 128:(occ + 1) * 128],
                            yt[fcp][:, t4 * 512:(t4 + 1) * 512],
                            start=(fcp == 0), stop=(fcp == PCH - 1))
                    nc.vector.tensor_scalar_add(
                        osb[:, t4 * 512:(t4 + 1) * 512], ps[:],
                        bout_sb[:, occ:occ + 1])
                nc.sync.dma_start(out_t[occ * 128:(occ + 1) * 128, :], osb[:])


def build_nc(T=2048):
    FC = 2 * CG // 128
    OCC = C // 128
    nc = bacc.Bacc("TRN2", target_bir_lowering=False, debug=False,
                   num_devices=N_CORES)
    x_t = nc.dram_tensor("x_t", [C, T], F32R, kind="ExternalInput")
    w_qk = nc.dram_tensor("w_qk", [C, 2 * CG], F32R, kind="ExternalInput")
    b_qk = nc.dram_tensor("b_qk", [128, FC], F32, kind="ExternalInput")
    w_v = nc.dram_tensor("w_v", [C, CG], F32R, kind="ExternalInput")
    ones_bf = nc.dram_tensor("ones_bf", [128, 8], BF16, kind="ExternalInput")
    w_p = nc.dram_tensor("w_p", [CG, C], BF16, kind="ExternalInput")
    b_out = nc.dram_tensor("b_out", [128, OCC], F32, kind="ExternalInput")
    out_t = nc.dram_tensor("out_t", [C, T], F32, kind="ExternalOutput")
    with tile.TileContext(nc) as tc:
        _body(tc, T, x_t.ap(), w_qk.ap(), b_qk.ap(), w_v.ap(), ones_bf.ap(),
              w_p.ap(), b_out.ap(), out_t.ap())
    nc.compile()
    return nc


def shard_inputs(sequences, w_attn, b_attn, w_proj, b_proj):
    """Build the 8 per-core input maps. Core index = b*2 + g."""
    sequences = np.asarray(sequences, dtype=np.float32)
    w_attn = np.asarray(w_attn, dtype=np.float32)
    b_attn = np.asarray(b_attn, dtype=np.float32)
    w_proj = np.asarray(w_proj, dtype=np.float32)
    b_proj = np.asarray(b_proj, dtype=np.float32)
    B = sequences.shape[0]
    in_maps = []
    for b in range(B):
        for g in range(2):
            qs = slice(g * CG, (g + 1) * CG)
            ks = slice(C + g * CG, C + (g + 1) * CG)
            vs = slice(2 * C + g * CG, 2 * C + (g + 1) * CG)
            in_maps.append({
                "x_t": np.ascontiguousarray(sequences[b].T),
                "w_qk": np.ascontiguousarray(
                    np.concatenate([w_attn[:, qs], w_attn[:, ks]], axis=1)),
                "b_qk": np.ascontiguousarray(
                    np.concatenate([b_attn[qs], b_attn[ks]])
                    .reshape(8, 128).T),
                "w_v": np.ascontiguousarray(w_attn[:, vs]),
                "ones_bf": np.ones((128, 8), ml_dtypes.bfloat16),
                "w_p": np.ascontiguousarray(w_proj[g * CG:(g + 1) * CG, :])
                    .astype(ml_dtypes.bfloat16),
                # softmax rows sum to 1, so the v-bias folds into the output
                # bias: y_g = attn@(x@w_v) @ w_p + (b_v@w_p [+ b_proj on g0])
                "b_out": np.ascontiguousarray(
                    (b_attn[vs] @ w_proj[g * CG:(g + 1) * CG, :]
                     + (b_proj if g == 0 else 0.0))
                    .astype(np.float32).reshape(8, 128).T),
            })
    return in_maps


def unshard_outputs(outs, B, T):
    """outs: list of 8 [C, T] partials, core index = b*2+g."""
    y = np.empty((B, T, C), np.float32)
    for b in range(B):
        y[b] = (outs[2 * b] + outs[2 * b + 1]).T
    return y


_NC_CACHE = {}


def kernel(sequences, w_attn, b_attn, w_proj, b_proj):
    sequences = np.asarray(sequences, dtype=np.float32)
    B, T, _ = sequences.shape
    in_maps = shard_inputs(sequences, w_attn, b_attn, w_proj, b_proj)
    if T not in _NC_CACHE:
        _NC_CACHE[T] = build_nc(T)
    nc = _NC_CACHE[T]
    res = run_bass_kernel_spmd(nc, in_maps, list(range(N_CORES)))
    outs = [res.results[i]["out_t"] for i in range(N_CORES)]
    return unshard_outputs(outs, B, T)


if __name__ == "__main__":
    rng = np.random.default_rng(0)
    B, T = 4, 2048
    seq = rng.standard_normal((B, T, C), dtype=np.float32)
    wa = rng.standard_normal((C, 3 * C), dtype=np.float32) / np.sqrt(C)
    ba = np.zeros(3 * C, np.float32)
    wp = rng.standard_normal((C, C), dtype=np.float32) / np.sqrt(C)
    bp = np.zeros(C, np.float32)
    y = kernel(seq, wa, ba, wp, bp)
    print(y.shape, y.dtype)


# revision 16
# speedup vs baseline: 1.2453x; 1.2453x over previous
"""Multi-head self-attention Trainium2 Bass kernel.

Problem: y = (softmax((x@Wq)(x@Wk)^T / sqrt(hd)) (x@Wv)) @ Wp + biases
with B=4, T=2048, C=1024, H=16, hd=64.

Sharding over 8 NeuronCores: (batch b in 0..3) x (head-group g in 0..1, 8
heads each) — tensor-parallel over heads, data-parallel over batch.  Each
core computes the attention for its batch and head group plus the partial
output projection restricted to its head group's features; the host sums
the two head-group partials per batch (the row-parallel TP reduction) and
transposes back.

Per-core device layout (T=2048, C=1024, Cg=512):
  x_t   [C, T]   f32  host-transposed activations
  Q^T,K^T [feat, tok] f32 computed as (w_qk chunk)^T @ x_t chunk
  V     [tok, feat]  bf16 computed as (x_t chunk)^T @ w_v chunk
  scores^T [k, q] = K_h Q_h^T / 8 accumulated in PSUM (contract d=64)
  E = exp(scores^T) via ScalarE with fused 0.125 scale, bf16
  U^T[d,q] + replicated denominator rows = [V_h | ones]^T @ E, one
    M=128 matmul per (head, kc) accumulated over kc in PSUM; normalize
    with DVE reciprocal + mult
  out^T[oc, tok] = (Y @ w_p)^T + b_out, DMA'd out; host sums pairs.
"""

import os

import numpy as np
import ml_dtypes

import concourse.bass as bass
import concourse.bacc as bacc
import concourse.tile as tile
from concourse import mybir
from concourse.bass_utils import run_bass_kernel_spmd

N_CORES = 8
C = 1024           # embed dim
H = 16             # total heads
HD = 64            # head dim
HPC = 8            # heads per core
CG = HPC * HD      # 512: per-core q/k/v feature width

F32 = mybir.dt.float32
F32R = mybir.dt.float32r
BF16 = mybir.dt.bfloat16


def _body(tc, T, x_t, w_qk, b_qk, w_v, ones_bf, w_p, b_out, out_t):
    nc = tc.nc
    KC = C // 128           # contraction chunks over C
    FC = 2 * CG // 128      # q||k feature chunks
    TC4 = T // 512          # token chunks of 512
    TC1 = T // 128          # key chunks of 128
    OCC = C // 128          # output channel chunks
    PCH = CG // 128         # proj contraction chunks (4)
    Exp = mybir.ActivationFunctionType.Exp
    Mult = mybir.AluOpType.mult

    with (
        tc.tile_pool(name="const", bufs=1) as constp,
        tc.tile_pool(name="persist", bufs=1) as pers,
    ):
        bqk_sb = constp.tile([128, FC], F32, tag="bqk")
        nc.sync.dma_start(bqk_sb[:], b_qk[:])
        bout_sb = constp.tile([128, OCC], F32, tag="bout")
        nc.sync.dma_start(bout_sb[:], b_out[:])

        xt = []
        for i in range(KC):
            t = pers.tile([128, T], F32R, tag=f"xt{i}")
            nc.sync.dma_start(t[:], x_t[i * 128:(i + 1) * 128, :])
            xt.append(t)

        # per-head blocks of 128 cols: [V_h (64) | ones (64)] so one M=128
        # matmul produces U^T on PSUM rows 0:64 and the softmax denominator
        # replicated on rows 64:128 -- full-width PE, no broadcast needed.
        v2 = [pers.tile([128, 2 * CG], BF16, tag=f"v2_{i}", name=f"v2_{i}")
              for i in range(TC1)]
        qkt = [pers.tile([128, T], F32R, tag=f"qkt{i}", name=f"qkt{i}")
               for i in range(FC)]
        yt = [pers.tile([128, T], BF16, tag=f"yt{i}", name=f"yt{i}")
              for i in range(PCH)]

        # ---- stage 1a: V = x @ w_v + b_v in [tok, feat] layout, bf16
        with (
            tc.tile_pool(name="wv", bufs=1) as wvp,
            tc.tile_pool(name="ps1", bufs=4, space="PSUM") as ps1,
        ):
            wv = []
            for kc in range(KC):
                t = wvp.tile([128, CG], F32R, tag=f"wv{kc}")
                nc.sync.dma_start(t[:], w_v[kc * 128:(kc + 1) * 128, :])
                wv.append(t)
            for tokc in range(TC1):
                vps = ps1.tile([128, CG], F32, tag="vps")
                for kc in range(KC):
                    nc.tensor.matmul(
                        vps[:], xt[kc][:, tokc * 128:(tokc + 1) * 128], wv[kc][:],
                        start=(kc == 0), stop=(kc == KC - 1))
                v2v = v2[tokc].rearrange("p (h c) -> p h c", c=2 * HD)
                nc.vector.tensor_copy(v2v[:, :, 0:HD],
                                      vps.rearrange("p (h c) -> p h c", c=HD))
                nc.sync.dma_start(
                    v2v[:, :, HD:2 * HD],
                    ones_bf.rearrange("p (h c) -> p h c", c=HD))

            # ---- stage 1b: QK^T = (x @ w_qk)^T + b in [feat, tok], f32
            # fc order pairs each Q chunk with its K chunk so stage 2's
            # first head pair can start after 2/8 of this stage.
            with tc.tile_pool(name="wqk", bufs=2) as wqkp:
                for fc in [x for p in range(FC // 2) for x in (p, FC // 2 + p)]:
                    wts = []
                    for kc in range(KC):
                        t = wqkp.tile([128, 128], F32R, tag=f"wqk{kc}")
                        nc.sync.dma_start(
                            t[:],
                            w_qk[kc * 128:(kc + 1) * 128, fc * 128:(fc + 1) * 128])
                        wts.append(t)
                    for t4 in range(TC4):
                        ps = ps1.tile([128, 512], F32, tag="qkps")
                        for kc in range(KC):
                            nc.tensor.matmul(
                                ps[:], wts[kc][:],
                                xt[kc][:, t4 * 512:(t4 + 1) * 512],
                                start=(kc == 0), stop=(kc == KC - 1))
                        nc.vector.tensor_scalar_add(
                            qkt[fc][:, t4 * 512:(t4 + 1) * 512], ps[:],
                            bqk_sb[:, fc:fc + 1])

        # ---- stage 2: head PAIRS, scores^T -> exp -> AV(+denominator) -> Y^T
        # The two heads of a pair sit at partitions 0:64 / 64:128 of one
        # qkt chunk, so their score matmuls (K=64) land on disjoint PE row
        # groups and run concurrently; the AV matmul and its ones-lhsT
        # denominator twin are col-tiled (M=64 each) and also concurrent.
        with (
            tc.tile_pool(name="e", bufs=3) as ep,
            tc.tile_pool(name="rec", bufs=2) as recp,
            tc.tile_pool(name="sps", bufs=2, space="PSUM") as spsp,
            tc.tile_pool(name="ups", bufs=2, space="PSUM") as upsp,
        ):
            nopair = bool(os.environ.get("KERNEL_NOPAIR"))
            for pair in range(HPC // 2):
                qt, kt = qkt[pair], qkt[FC // 2 + pair]
                for qc in range(T // 512):
                    q0 = qc * 512
                    heads = ([(0,), (1,)] if nopair else [(0, 1)])
                    for group in heads:
                        upsG = {s: upsp.tile([128, 512], F32, tag=f"ups{s}",
                                             name=f"ups{s}_{pair}_{qc}")
                                for s in group}
                        for kc in range(TC1):
                            first, last = (kc == 0), (kc == TC1 - 1)
                            ets = {}
                            for s in group:
                                po = s * 64
                                sps = spsp.tile(
                                    [128, 512], F32, tag=f"sps{s}",
                                    name=f"sps{s}_{pair}_{qc}_{kc}")
                                nc.tensor.matmul(
                                    sps[:],
                                    kt[po:po + 64, kc * 128:(kc + 1) * 128],
                                    qt[po:po + 64, q0:q0 + 512],
                                    start=True, stop=True)
                                et = ep.tile([128, 512], BF16, tag=f"et{s}",
                                             name=f"et{s}_{pair}_{qc}_{kc}")
                                nc.scalar.activation(et[:], sps[:], Exp,
                                                     scale=0.125)
                                ets[s] = et
                            for s in group:
                                h = 2 * pair + s
                                nc.tensor.matmul(
                                    upsG[s][:],
                                    v2[kc][:, h * 128:(h + 1) * 128],
                                    ets[s][:],
                                    start=first, stop=last)
                        for s in group:
                            po, ups = s * 64, upsG[s]
                            rec = recp.tile([64, 512], F32, tag=f"rec{s}",
                                            name=f"rec{s}_{pair}_{qc}")
                            nc.vector.reciprocal(rec[:], ups[64:128, :])
                            nc.vector.tensor_tensor(
                                yt[pair][po:po + 64, q0:q0 + 512],
                                ups[0:64, :], rec[:], op=Mult)

        # ---- stage 3: out^T = (Y @ w_p)^T + b_out
        with (
            tc.tile_pool(name="wp", bufs=1) as wpp,
            tc.tile_pool(name="outp", bufs=2) as outp,
            tc.tile_pool(name="ps3", bufs=4, space="PSUM") as ps3,
        ):
            wp = []
            for fcp in range(PCH):
                t = wpp.tile([128, C], BF16, tag=f"wp{fcp}")
                nc.sync.dma_start(t[:], w_p[fcp * 128:(fcp + 1) * 128, :])
                wp.append(t)
            for occ in range(OCC):
                osb = outp.tile([128, T], F32, tag="osb")
                for t4 in range(TC4):
                    ps = ps3.tile([128, 512], F32, tag="ps3")
                    for fcp in range(PCH):
                        nc.tensor.matmul(
                            ps[:], wp[fcp][:, occ * 128:(occ + 1) * 128],
                            yt[fcp][:, t4 * 512:(t4 + 1) * 512],
                            start=(fcp == 0), stop=(fcp == PCH - 1))
                    nc.vector.tensor_scalar_add(
                        osb[:, t4 * 512:(t4 + 1) * 512], ps[:],
                        bout_sb[:, occ:occ + 1])
                nc.sync.dma_start(out_t[occ * 128:(occ + 1) * 128, :], osb[:])


def build_nc(T=2048):
    FC = 2 * CG // 128
    OCC = C // 128
    nc = bacc.Bacc("TRN2", target_bir_lowering=False, debug=False,
                   num_devices=N_CORES)
    x_t = nc.dram_tensor("x_t", [C, T], F32R, kind="ExternalInput")
    w_qk = nc.dram_tensor("w_qk", [C, 2 * CG], F32R, kind="ExternalInput")
    b_qk = nc.dram_tensor("b_qk", [128, FC], F32, kind="ExternalInput")
    w_v = nc.dram_tensor("w_v", [C, CG], F32R, kind="ExternalInput")
    ones_bf = nc.dram_tensor("ones_bf", [128, CG], BF16, kind="ExternalInput")
    w_p = nc.dram_tensor("w_p", [CG, C], BF16, kind="ExternalInput")
    b_out = nc.dram_tensor("b_out", [128, OCC], F32, kind="ExternalInput")
    out_t = nc.dram_tensor("out_t", [C, T], F32, kind="ExternalOutput")
    with tile.TileContext(nc) as tc:
        _body(tc, T, x_t.ap(), w_qk.ap(), b_qk.ap(), w_v.ap(), ones_bf.ap(),
              w_p.ap(), b_out.ap(), out_t.ap())
    nc.compile()
    return nc


def shard_inputs(sequences, w_attn, b_attn, w_proj, b_proj):
    """Build the 8 per-core input maps. Core index = b*2 + g."""
    sequences = np.asarray(sequences, dtype=np.float32)
    w_attn = np.asarray(w_attn, dtype=np.float32)
    b_attn = np.asarray(b_attn, dtype=np.float32)
    w_proj = np.asarray(w_proj, dtype=np.float32)
    b_proj = np.asarray(b_proj, dtype=np.float32)
    B = sequences.shape[0]
    in_maps = []
    for b in range(B):
        for g in range(2):
            qs = slice(g * CG, (g + 1) * CG)
            ks = slice(C + g * CG, C + (g + 1) * CG)
            vs = slice(2 * C + g * CG, 2 * C + (g + 1) * CG)
            in_maps.append({
                "x_t": np.ascontiguousarray(sequences[b].T),
                "w_qk": np.ascontiguousarray(
                    np.concatenate([w_attn[:, qs], w_attn[:, ks]], axis=1)),
                "b_qk": np.ascontiguousarray(
                    np.concatenate([b_attn[qs], b_attn[ks]])
                    .reshape(8, 128).T),
                "w_v": np.ascontiguousarray(w_attn[:, vs]),
                "ones_bf": np.ones((128, CG), ml_dtypes.bfloat16),
                "w_p": np.ascontiguousarray(w_proj[g * CG:(g + 1) * CG, :])
                    .astype(ml_dtypes.bfloat16),
                # softmax rows sum to 1, so the v-bias folds into the output
                # bias: y_g = attn@(x@w_v) @ w_p + (b_v@w_p [+ b_proj on g0])
                "b_out": np.ascontiguousarray(
                    (b_attn[vs] @ w_proj[g * CG:(g + 1) * CG, :]
                     + (b_proj if g == 0 else 0.0))
                    .astype(np.float32).reshape(8, 128).T),
            })
    return in_maps


def unshard_outputs(outs, B, T):
    """outs: list of 8 [C, T] partials, core index = b*2+g."""
    y = np.empty((B, T, C), np.float32)
    for b in range(B):
        y[b] = (outs[2 * b] + outs[2 * b + 1]).T
    return y


_NC_CACHE = {}


def kernel(sequences, w_attn, b_attn, w_proj, b_proj):
    sequences = np.asarray(sequences, dtype=np.float32)
    B, T, _ = sequences.shape
    in_maps = shard_inputs(sequences, w_attn, b_attn, w_proj, b_proj)
    if T not in _NC_CACHE:
        _NC_CACHE[T] = build_nc(T)
    nc = _NC_CACHE[T]
    res = run_bass_kernel_spmd(nc, in_maps, list(range(N_CORES)))
    outs = [res.results[i]["out_t"] for i in range(N_CORES)]
    return unshard_outputs(outs, B, T)


if __name__ == "__main__":
    rng = np.random.default_rng(0)
    B, T = 4, 2048
    seq = rng.standard_normal((B, T, C), dtype=np.float32)
    wa = rng.standard_normal((C, 3 * C), dtype=np.float32) / np.sqrt(C)
    ba = np.zeros(3 * C, np.float32)
    wp = rng.standard_normal((C, C), dtype=np.float32) / np.sqrt(C)
    bp = np.zeros(C, np.float32)
    y = kernel(seq, wa, ba, wp, bp)
    print(y.shape, y.dtype)
